# revision 3
# baseline (speedup 1.0000x reference)
import numpy as np

B = 128
FEAT = 64
LATENT = 512
OUT_F = 6144  # NUM_POINTS * 3
EPS = 1e-5
N_CORES = 8
SEGS_PER_CORE = 16
S_PAD = 8192
NW3 = OUT_F // N_CORES  # 768 output cols per core
F16MIN = np.float16(-65504.0)

# zn128 column slot j -> global segment id
SIG = np.array(
    [16 * ((j % 64) // 8) + 8 * (j // 64) + (j % 8) for j in range(128)],
    dtype=np.int64,
)

_CACHE = {}


def build_nc():
    from concourse import bass, bacc, tile

    mybir = bass.mybir
    f32 = mybir.dt.float32
    f16 = mybir.dt.float16
    f32r = mybir.dt.float32r
    AF = mybir.ActivationFunctionType
    ALU = mybir.AluOpType

    nc = bacc.Bacc("TRN2", num_devices=N_CORES)
    xt_d = nc.declare_dram_parameter("xt", [128, 8, S_PAD], f16, isOutput=False)
    wp_d = nc.declare_dram_parameter("wp2", [128, LATENT], f16, isOutput=False)
    bp_d = nc.declare_dram_parameter("bp", [128, 4], f32, isOutput=False)
    w1_d = nc.declare_dram_parameter("w1p", [128, 1024], f16, isOutput=False)
    b1_d = nc.declare_dram_parameter("b1p", [128, 2], f32, isOutput=False)
    w2_d = nc.declare_dram_parameter("w2p", [128, 1024], f16, isOutput=False)
    b2_d = nc.declare_dram_parameter("b2p", [128, 4], f32, isOutput=False)
    w3_d = nc.declare_dram_parameter("w3p", [128, 4, NW3], f16, isOutput=False)
    selT_d = nc.declare_dram_parameter("selT", [128, 2], f32, isOutput=False)
    sel_d = nc.declare_dram_parameter("sel", [2, 128], f32, isOutput=False)
    out_d = nc.declare_dram_parameter("out", [128, NW3], f16, isOutput=True)

    with tile.TileContext(nc) as tc:
        with (
            tc.tile_pool(name="wpool", bufs=1) as wpool,
            tc.tile_pool(name="fpool", bufs=4) as fpool,
            tc.tile_pool(name="spool", bufs=1) as spool,
            tc.tile_pool(name="dram", bufs=1, space="DRAM") as dpool,
            tc.tile_pool(name="ps_s", bufs=1, space=bass.MemorySpace.PSUM) as ps_s,
            tc.tile_pool(name="ps_b", bufs=1, space=bass.MemorySpace.PSUM) as ps_b,
            tc.tile_pool(name="ps_m", bufs=3, space=bass.MemorySpace.PSUM) as ps_m,
            tc.tile_pool(name="ps_o", bufs=2, space=bass.MemorySpace.PSUM) as ps_o,
        ):
            wp = wpool.tile([128, LATENT], f16)
            bp = wpool.tile([128, 4], f32)
            w1 = wpool.tile([128, 1024], f16)
            b1 = wpool.tile([128, 2], f32)
            w2 = wpool.tile([128, 1024], f16)
            b2 = wpool.tile([128, 4], f32)
            w3t = wpool.tile([128, 4, NW3], f16)
            # small weights on the gpsimd SWDGE queue: keeps both HW queues
            # free for feat streaming from t=0
            for t, d in (
                (wp, wp_d), (bp, bp_d), (w1, w1_d), (b1, b1_d),
                (w2, w2_d), (b2, b2_d), (w3t, w3_d),
            ):
                nc.gpsimd.dma_start(t[:], d[:])

            # group-selector matrices for the two-group LayerNorm
            sel2T = wpool.tile([128, 2], f32)  # sel2T[p, j] = (p // 64 == j)
            sel2 = wpool.tile([2, 128], f32)  # sel2[j, p] = (p // 64 == j)
            eps_t = wpool.tile([2, 1], f32)
            scr = wpool.tile([2, 1], f32)
            nc.gpsimd.dma_start(sel2T[:], selT_d[:])
            nc.gpsimd.dma_start(sel2[:], sel_d[:])
            nc.vector.memset(eps_t[:], EPS)
            # hoist the Sqrt activation-table load out of the tail
            nc.scalar.activation(scr[:], eps_t[:], AF.Sqrt)

            val_h = spool.tile([128, 8], f16)
            valw = spool.tile([128, 6], f16)
            val32 = spool.tile([128, 8], f32)
            val_all = spool.tile([128, 64], f32)
            mu2 = spool.tile([2, 64], f32)
            std = spool.tile([2, 64], f32)
            rstd = spool.tile([2, 64], f32)
            zc = spool.tile([128, 64], f32)
            zsq = spool.tile([128, 64], f32)
            zn128 = spool.tile([128, 128], f16)
            lat = spool.tile([128, 512], f16)
            h1 = spool.tile([128, 256], f16)
            h2 = spool.tile([128, 512], f16)
            out_sb = spool.tile([128, NW3], f16)
            nc.vector.memset(zn128[:], 0.0)

            in_b = dpool.tile([128, 8], f32)
            out_b = dpool.tile([8, 128, 8], f32)

            # --- segment max pooling: even tiles on SP queue, odd on Act.
            # last tile per queue split 4096+2048+2048 to shorten the tail ---
            for t in range(6):
                ft = fpool.tile([128, S_PAD], f16, name="ft")
                eng = nc.sync if t % 2 == 0 else nc.scalar
                eng.dma_start(ft[:], xt_d[:, t, :])
                nc.vector.reduce_max(
                    val_h[:, t : t + 1], ft[:], axis=mybir.AxisListType.X
                )
            chunks = [(0, 4096), (4096, 2048), (6144, 2048)]
            for i, (o, L) in enumerate(chunks):
                for t in (6, 7):
                    fh = fpool.tile([128, L], f16, name="ft")
                    eng = nc.sync if t % 2 == 0 else nc.scalar
                    eng.dma_start(fh[:], xt_d[:, t, o : o + L])
                    nc.vector.reduce_max(
                        valw[:, 3 * (t - 6) + i : 3 * (t - 6) + i + 1], fh[:],
                        axis=mybir.AxisListType.X,
                    )
            nc.vector.reduce_max(
                val_h[:, 6:7], valw[:, 0:3], axis=mybir.AxisListType.X
            )
            nc.vector.reduce_max(
                val_h[:, 7:8], valw[:, 3:6], axis=mybir.AxisListType.X
            )
            nc.vector.tensor_copy(val32[:], val_h[:])

            # --- PE warm-up burst, gated on the 6th tile's reduce so it runs
            # during the last ~5us of the stream and leaves HAM at K=8/8 for
            # the tail matmuls. gate creates the RAW dep; PE FIFO order gates
            # the rest of the burst (and the LN/MLP matmuls) behind it. ---
            gate = ps_s.tile([2, 1], f32, name="gate")
            nc.tensor.matmul(
                gate[:], wp[:, 0:2], val_h[:, 5:6], start=True, stop=True
            )
            wps = ps_o.tile([128, 512], f32, name="pso")
            for _ in range(12):
                nc.tensor.matmul(
                    wps[:], wp[:, 0:128], w3t[:, 0, 0:512], start=True, stop=True
                )

            # --- AllGather pooled vals: [128, 8] f32 per core -> all cores
            # hold all 128 segments as val_all[p, 8c + t] ---
            nc.sync.dma_start(in_b[:], val32[:])
            nc.gpsimd.collective_compute(
                "AllGather",
                ALU.bypass,
                replica_groups=[list(range(N_CORES))],
                ins=[in_b[:]],
                outs=[out_b[:]],
            )
            nc.scalar.dma_start(val_all[:], out_b[:].transpose([1, 0, 2]))

            # --- LayerNorm per (group, col) on val_all [128, 64] ---
            red = ps_s.tile([2, 64], f32, name="red")
            nc.tensor.matmul(red[:], sel2T[:], val_all[:], start=True, stop=True)
            nc.scalar.mul(mu2[:], red[:], 1.0 / FEAT)
            bc = ps_b.tile([128, 64], f32, name="bc")
            nc.tensor.matmul(bc[:], sel2[:], mu2[:], start=True, stop=True)
            nc.vector.tensor_tensor(zc[:], val_all[:], bc[:], op=ALU.subtract)
            nc.vector.tensor_tensor(zsq[:], zc[:], zc[:], op=ALU.mult)
            red2 = ps_s.tile([2, 64], f32, name="red")
            nc.tensor.matmul(red2[:], sel2T[:], zsq[:], start=True, stop=True)
            nc.scalar.activation(
                std[:], red2[:], AF.Sqrt, bias=eps_t[:], scale=1.0 / FEAT
            )
            nc.vector.reciprocal(rstd[:], std[:])
            bc2 = ps_b.tile([128, 64], f32, name="bc")
            nc.tensor.matmul(bc2[:], sel2[:], rstd[:], start=True, stop=True)
            # scatter normalized groups into disjoint columns of zn128
            nc.vector.tensor_tensor(
                zn128[0:64, 0:64], zc[0:64, :], bc2[0:64, :], op=ALU.mult
            )
            nc.vector.tensor_tensor(
                zn128[64:128, 64:128], zc[64:128, :], bc2[64:128, :], op=ALU.mult
            )

            # --- proj (ln affine folded into wp/bp): lat[128m+p, slot] ---
            for m in range(4):
                ps = ps_m.tile([128, 128], f32)
                nc.tensor.matmul(
                    ps[:], wp[:, 128 * m : 128 * (m + 1)], zn128[:],
                    start=True, stop=True,
                )
                nc.vector.tensor_scalar(
                    lat[:, 128 * m : 128 * (m + 1)], ps[:], bp[:, m : m + 1],
                    None, op0=ALU.add,
                )

            # --- h1 = relu(latent @ w1 + b1), transposed ---
            for n in range(2):
                ps = ps_m.tile([128, 128], f32)
                for k in range(4):
                    nc.tensor.matmul(
                        ps[:],
                        w1[:, (k * 2 + n) * 128 : (k * 2 + n + 1) * 128],
                        lat[:, 128 * k : 128 * (k + 1)],
                        start=(k == 0), stop=(k == 3),
                    )
                nc.vector.tensor_scalar(
                    h1[:, 128 * n : 128 * (n + 1)], ps[:], b1[:, n : n + 1],
                    0.0, op0=ALU.add, op1=ALU.max,
                )

            # --- h2 = relu(h1 @ w2 + b2), transposed ---
            for n in range(4):
                ps = ps_m.tile([128, 128], f32)
                for k in range(2):
                    nc.tensor.matmul(
                        ps[:],
                        w2[:, (k * 4 + n) * 128 : (k * 4 + n + 1) * 128],
                        h1[:, 128 * k : 128 * (k + 1)],
                        start=(k == 0), stop=(k == 1),
                    )
                nc.vector.tensor_scalar(
                    h2[:, 128 * n : 128 * (n + 1)], ps[:], b2[:, n : n + 1],
                    0.0, op0=ALU.add, op1=ALU.max,
                )

            # --- out[slot, n] = h2.T @ w3 shard (this core's 768 cols) ---
            for half in range(2):
                n0 = 384 * half
                po = ps_o.tile([128, 384], f32, name="pso")
                for k in range(4):
                    nc.tensor.matmul(
                        po[:],
                        h2[:, 128 * k : 128 * (k + 1)],
                        w3t[:, k, n0 : n0 + 384],
                        start=(k == 0), stop=(k == 3),
                    )
                nc.vector.tensor_copy(out_sb[:, n0 : n0 + 384], po[:])
                nc.sync.dma_start(
                    out_d[:, n0 : n0 + 384], out_sb[:, n0 : n0 + 384]
                )

    nc.finalize()
    return nc


def pack_weights(ln_g, ln_b, proj_w, proj_b, w1, b1, w2, b2, w3, b3):
    c = np.ascontiguousarray
    wp = (ln_g[:, None] * proj_w).astype(np.float32)  # [64, 512]
    bpv = (ln_b.astype(np.float64) @ proj_w.astype(np.float64)).astype(np.float32) + proj_b
    shared = {
        "wp2": c(np.vstack([wp, wp]).astype(np.float16)),
        "bp": c(bpv.reshape(4, 128).T),
        "w1p": c(
            w1.reshape(4, 128, 2, 128).transpose(1, 0, 2, 3).reshape(128, 1024)
            .astype(np.float16)
        ),
        "b1p": c(b1.reshape(2, 128).T),
        "w2p": c(
            w2.reshape(2, 128, 4, 128).transpose(1, 0, 2, 3).reshape(128, 1024)
            .astype(np.float16)
        ),
        "b2p": c(b2.reshape(4, 128).T),
        "selT": c(np.repeat(np.eye(2, dtype=np.float32), 64, axis=0)),
        "sel": c(np.repeat(np.eye(2, dtype=np.float32), 64, axis=1)),
    }
    # per-core w3 column shard: w3sh[p, k, n] = w3[128k + p, NW3*c + n]
    w3h = w3.astype(np.float16)
    w3ps = [
        c(w3h[:, NW3 * cc : NW3 * (cc + 1)].reshape(4, 128, NW3).transpose(1, 0, 2))
        for cc in range(N_CORES)
    ]
    return shared, w3ps


def pack_feat_core(feat16, feat32, bounds, c):
    xt = np.full((128, 8, S_PAD), F16MIN, np.float16)
    for sl in range(SEGS_PER_CORE):
        seg = c * SEGS_PER_CORE + sl
        a, b = bounds[seg], bounds[seg + 1]
        L = b - a
        if L > S_PAD:
            blk = np.concatenate(
                [
                    feat16[a : a + S_PAD - 1],
                    feat32[a + S_PAD - 1 : b].max(0, keepdims=True).astype(np.float16),
                ],
                0,
            )
            L = S_PAD
        else:
            blk = feat16[a:b]
        g, t = divmod(sl, 8)
        if L > 0:
            xt[g * 64 : (g + 1) * 64, t, :L] = blk.T
    return xt


def make_in_maps(inputs):
    feat32 = np.asarray(inputs["feat"], dtype=np.float32)
    feat16 = feat32.astype(np.float16)
    batch = np.asarray(inputs["batch"])
    shared, w3ps = pack_weights(
        *(np.asarray(inputs[k], dtype=np.float32) for k in
          ("ln_g", "ln_b", "proj_w", "proj_b", "w1", "b1", "w2", "b2", "w3", "b3"))
    )
    bounds = np.searchsorted(batch, np.arange(B + 1))
    return [
        {"xt": pack_feat_core(feat16, feat32, bounds, c), "w3p": w3ps[c], **shared}
        for c in range(N_CORES)
    ]


def kernel(**inputs):
    from concourse.bass_utils import run_bass_kernel_spmd

    if "nc" not in _CACHE:
        _CACHE["nc"] = build_nc()
    nc = _CACHE["nc"]

    in_maps = make_in_maps(inputs)
    res = run_bass_kernel_spmd(nc, in_maps, list(range(N_CORES)))

    out = np.empty((B, OUT_F), np.float32)
    for c in range(N_CORES):
        out[SIG, NW3 * c : NW3 * (c + 1)] = res.results[c]["out"].astype(np.float32)
    out += np.asarray(inputs["b3"], dtype=np.float32)[None, :]
    return out.reshape(B, 2048, 3)


# revision 5
# speedup vs baseline: 1.2065x; 1.2065x over previous
import numpy as np

B = 128
FEAT = 64
LATENT = 512
OUT_F = 6144  # NUM_POINTS * 3
EPS = 1e-5
N_CORES = 8
SEGS_PER_CORE = 16
S_PAD = 8192
F16MIN = np.float16(-65504.0)
N_CAST = 2  # leading tile slots cast fp16->f32 during DMA (SWDGE) so the
            # DVE reduces them at 2 elem/cyc; fp16 reduce is only 1 elem/cyc

_CACHE = {}


def build_nc():
    from concourse import bass, bacc, tile

    mybir = bass.mybir
    f32 = mybir.dt.float32
    f16 = mybir.dt.float16
    AF = mybir.ActivationFunctionType
    ALU = mybir.AluOpType

    nc = bacc.Bacc("TRN2")
    xt_d = nc.declare_dram_parameter("xt", [128, 8, S_PAD], f16, isOutput=False)
    wp_d = nc.declare_dram_parameter("wp2", [128, LATENT], f16, isOutput=False)
    bp_d = nc.declare_dram_parameter("bp", [128, 4], f32, isOutput=False)
    w1_d = nc.declare_dram_parameter("w1p", [128, 1024], f16, isOutput=False)
    b1_d = nc.declare_dram_parameter("b1p", [128, 2], f32, isOutput=False)
    w2_d = nc.declare_dram_parameter("w2p", [128, 1024], f16, isOutput=False)
    b2_d = nc.declare_dram_parameter("b2p", [128, 4], f32, isOutput=False)
    w3_d = nc.declare_dram_parameter("w3p", [128, 3, 4, 2048], f16, isOutput=False)
    selT_d = nc.declare_dram_parameter("selT", [128, 2], f32, isOutput=False)
    sel_d = nc.declare_dram_parameter("sel", [2, 128], f32, isOutput=False)
    out_d = nc.declare_dram_parameter("out", [16, OUT_F], f16, isOutput=True)

    with tile.TileContext(nc) as tc:
        with (
            tc.tile_pool(name="wpool", bufs=1) as wpool,
            tc.tile_pool(name="fpool", bufs=3) as fpool,
            tc.tile_pool(name="cpool", bufs=2) as cpool,
            tc.tile_pool(name="spool", bufs=1) as spool,
            tc.tile_pool(name="ps_s", bufs=1, space=bass.MemorySpace.PSUM) as ps_s,
            tc.tile_pool(name="ps_b", bufs=1, space=bass.MemorySpace.PSUM) as ps_b,
            tc.tile_pool(name="ps_m", bufs=3, space=bass.MemorySpace.PSUM) as ps_m,
            tc.tile_pool(name="ps_o", bufs=2, space=bass.MemorySpace.PSUM) as ps_o,
        ):
            wp = wpool.tile([128, LATENT], f16)
            bp = wpool.tile([128, 4], f32)
            w1 = wpool.tile([128, 1024], f16)
            b1 = wpool.tile([128, 2], f32)
            w2 = wpool.tile([128, 1024], f16)
            b2 = wpool.tile([128, 4], f32)
            sel2T = wpool.tile([128, 2], f32)
            sel2 = wpool.tile([2, 128], f32)
            eps_t = wpool.tile([2, 1], f32)
            scr = wpool.tile([2, 1], f32)
            w3t = wpool.tile([128, 3, 4, 2048], f16)

            # cast tiles 0..N_CAST-1 go first on the SWDGE queue so the DVE
            # can start reducing them at f32 rate while the HWDGE queues
            # stream the fp16 tiles
            cast_tiles = []
            for t in range(N_CAST):
                cf = cpool.tile([128, S_PAD], f32, name="cf")
                nc.gpsimd.dma_start(cf[:], xt_d[:, t, :])
                cast_tiles.append(cf)

            nc.gpsimd.dma_start(sel2T[:], selT_d[:])
            nc.gpsimd.dma_start(sel2[:], sel_d[:])
            for t, d in (
                (wp, wp_d), (bp, bp_d), (w1, w1_d), (b1, b1_d),
                (w2, w2_d), (b2, b2_d),
            ):
                nc.gpsimd.dma_start(t[:], d[:])
            nc.gpsimd.dma_start(w3t[:], w3_d[:])

            nc.vector.memset(eps_t[:], EPS)
            # hoist the Sqrt activation-table load out of the tail
            nc.scalar.activation(scr[:], eps_t[:], AF.Sqrt)

            val32 = spool.tile([128, 8], f32)
            val_h = spool.tile([128, 6], f16)
            valw = spool.tile([128, 6], f16)
            mu2 = spool.tile([2, 8], f32)
            std = spool.tile([2, 8], f32)
            rstd = spool.tile([2, 8], f32)
            zc = spool.tile([128, 8], f32)
            zsq = spool.tile([128, 8], f32)
            zn16 = spool.tile([128, 16], f16)
            lat = spool.tile([128, 64], f16)
            h1 = spool.tile([128, 32], f16)
            h2 = spool.tile([128, 64], f16)
            out_sb = spool.tile([16, OUT_F], f16)
            nc.vector.memset(zn16[:], 0.0)

            # --- segment max pooling ---
            # slots 0..1: f32 cast tiles (SWDGE), reduce at 2 elem/cyc
            for t in range(N_CAST):
                nc.vector.reduce_max(
                    val32[:, t : t + 1], cast_tiles[t][:], axis=mybir.AxisListType.X
                )
            # slots 2..5: fp16 on the two HWDGE queues
            for t in range(N_CAST, 6):
                ft = fpool.tile([128, S_PAD], f16, name="ft")
                eng = nc.sync if t % 2 == 0 else nc.scalar
                eng.dma_start(ft[:], xt_d[:, t, :])
                nc.vector.reduce_max(
                    val_h[:, t - 2 : t - 1], ft[:], axis=mybir.AxisListType.X
                )
            # slots 6,7: chunked 4096+2048+2048 to shorten the reduce tail
            chunks = [(0, 4096), (4096, 2048), (6144, 2048)]
            for i, (o, L) in enumerate(chunks):
                for t in (6, 7):
                    fh = fpool.tile([128, L], f16, name="ft")
                    eng = nc.sync if t % 2 == 0 else nc.scalar
                    eng.dma_start(fh[:], xt_d[:, t, o : o + L])
                    nc.vector.reduce_max(
                        valw[:, 3 * (t - 6) + i : 3 * (t - 6) + i + 1], fh[:],
                        axis=mybir.AxisListType.X,
                    )
            nc.vector.reduce_max(
                val_h[:, 4:5], valw[:, 0:3], axis=mybir.AxisListType.X
            )
            nc.vector.reduce_max(
                val_h[:, 5:6], valw[:, 3:6], axis=mybir.AxisListType.X
            )
            # val32 cols 2..7 <- val_h cols 0..5 (fp16 -> f32)
            nc.vector.tensor_copy(val32[:, 2:8], val_h[:])

            # --- PE warm-up burst, gated on slot 5's reduce (lands a few us
            # before stream end) so HAM is at K=8/8 for the tail matmuls.
            # PE FIFO order gates the burst + all later matmuls behind it. ---
            gate = ps_s.tile([2, 1], f32, name="gate")
            nc.tensor.matmul(
                gate[:], wp[:, 0:2], val_h[:, 3:4], start=True, stop=True
            )
            wps = ps_o.tile([16, 512], f32, name="pso")
            for _ in range(12):
                nc.tensor.matmul(
                    wps[:], w3t[:, 0, 0, 0:16], w3t[:, 0, 1, 0:512],
                    start=True, stop=True,
                )

            # --- LayerNorm per (group, col) on val32 [128, 8] ---
            red = ps_s.tile([2, 8], f32, name="red")
            nc.tensor.matmul(red[:], sel2T[:], val32[:], start=True, stop=True)
            nc.scalar.mul(mu2[:], red[:], 1.0 / FEAT)
            bc = ps_b.tile([128, 8], f32, name="bc")
            nc.tensor.matmul(bc[:], sel2[:], mu2[:], start=True, stop=True)
            nc.vector.tensor_tensor(zc[:], val32[:], bc[:], op=ALU.subtract)
            nc.vector.tensor_tensor(zsq[:], zc[:], zc[:], op=ALU.mult)
            red2 = ps_s.tile([2, 8], f32, name="red")
            nc.tensor.matmul(red2[:], sel2T[:], zsq[:], start=True, stop=True)
            nc.scalar.activation(
                std[:], red2[:], AF.Sqrt, bias=eps_t[:], scale=1.0 / FEAT
            )
            nc.vector.reciprocal(rstd[:], std[:])
            bc2 = ps_b.tile([128, 8], f32, name="bc")
            nc.tensor.matmul(bc2[:], sel2[:], rstd[:], start=True, stop=True)
            nc.vector.tensor_tensor(
                zn16[0:64, 0:8], zc[0:64, :], bc2[0:64, :], op=ALU.mult
            )
            nc.vector.tensor_tensor(
                zn16[64:128, 8:16], zc[64:128, :], bc2[64:128, :], op=ALU.mult
            )

            # --- proj (ln affine folded into wp/bp): lat[128m+p, s] ---
            for m in range(4):
                ps = ps_m.tile([128, 16], f32)
                nc.tensor.matmul(
                    ps[:], wp[:, 128 * m : 128 * (m + 1)], zn16[:],
                    start=True, stop=True,
                )
                nc.vector.tensor_scalar(
                    lat[:, 16 * m : 16 * (m + 1)], ps[:], bp[:, m : m + 1],
                    None, op0=ALU.add,
                )

            # --- h1 = relu(latent @ w1 + b1), transposed ---
            for n in range(2):
                ps = ps_m.tile([128, 16], f32)
                for k in range(4):
                    nc.tensor.matmul(
                        ps[:],
                        w1[:, (k * 2 + n) * 128 : (k * 2 + n + 1) * 128],
                        lat[:, 16 * k : 16 * (k + 1)],
                        start=(k == 0), stop=(k == 3),
                    )
                nc.vector.tensor_scalar(
                    h1[:, 16 * n : 16 * (n + 1)], ps[:], b1[:, n : n + 1],
                    0.0, op0=ALU.add, op1=ALU.max,
                )

            # --- h2 = relu(h1 @ w2 + b2), transposed ---
            for n in range(4):
                ps = ps_m.tile([128, 16], f32)
                for k in range(2):
                    nc.tensor.matmul(
                        ps[:],
                        w2[:, (k * 4 + n) * 128 : (k * 4 + n + 1) * 128],
                        h1[:, 16 * k : 16 * (k + 1)],
                        start=(k == 0), stop=(k == 1),
                    )
                nc.vector.tensor_scalar(
                    h2[:, 16 * n : 16 * (n + 1)], ps[:], b2[:, n : n + 1],
                    0.0, op0=ALU.add, op1=ALU.max,
                )

            # --- out[s, n] = h2.T @ w3: h2 as stationary lhsT (fp16) ---
            for g in range(3):
                for sub in range(4):
                    n0 = 2048 * g + 512 * sub
                    ps = ps_o.tile([16, 512], f32, name="pso")
                    for k in range(4):
                        nc.tensor.matmul(
                            ps[:],
                            h2[:, 16 * k : 16 * (k + 1)],
                            w3t[:, g, k, 512 * sub : 512 * (sub + 1)],
                            start=(k == 0), stop=(k == 3),
                        )
                    nc.vector.tensor_copy(out_sb[:, n0 : n0 + 512], ps[:])
                nc.sync.dma_start(
                    out_d[:, 2048 * g : 2048 * (g + 1)],
                    out_sb[:, 2048 * g : 2048 * (g + 1)],
                )

    nc.finalize()
    return nc


def pack_weights(ln_g, ln_b, proj_w, proj_b, w1, b1, w2, b2, w3, b3):
    c = np.ascontiguousarray
    wp = (ln_g[:, None] * proj_w).astype(np.float32)  # [64, 512]
    bpv = (ln_b.astype(np.float64) @ proj_w.astype(np.float64)).astype(np.float32) + proj_b
    return {
        "wp2": c(np.vstack([wp, wp]).astype(np.float16)),
        "bp": c(bpv.reshape(4, 128).T),
        "w1p": c(
            w1.reshape(4, 128, 2, 128).transpose(1, 0, 2, 3).reshape(128, 1024)
            .astype(np.float16)
        ),
        "b1p": c(b1.reshape(2, 128).T),
        "w2p": c(
            w2.reshape(2, 128, 4, 128).transpose(1, 0, 2, 3).reshape(128, 1024)
            .astype(np.float16)
        ),
        "b2p": c(b2.reshape(4, 128).T),
        "w3p": c(
            w3.reshape(4, 128, 3, 2048).transpose(1, 2, 0, 3).astype(np.float16)
        ),
        "selT": c(np.repeat(np.eye(2, dtype=np.float32), 64, axis=0)),
        "sel": c(np.repeat(np.eye(2, dtype=np.float32), 64, axis=1)),
    }


def pack_feat_core(feat16, feat32, bounds, c):
    xt = np.full((128, 8, S_PAD), F16MIN, np.float16)
    for sl in range(SEGS_PER_CORE):
        seg = c * SEGS_PER_CORE + sl
        a, b = bounds[seg], bounds[seg + 1]
        L = b - a
        if L > S_PAD:
            blk = np.concatenate(
                [
                    feat16[a : a + S_PAD - 1],
                    feat32[a + S_PAD - 1 : b].max(0, keepdims=True).astype(np.float16),
                ],
                0,
            )
            L = S_PAD
        else:
            blk = feat16[a:b]
        g, t = divmod(sl, 8)
        if L > 0:
            xt[g * 64 : (g + 1) * 64, t, :L] = blk.T
    return xt


def make_in_maps(inputs):
    feat32 = np.asarray(inputs["feat"], dtype=np.float32)
    feat16 = feat32.astype(np.float16)
    batch = np.asarray(inputs["batch"])
    wdict = pack_weights(
        *(np.asarray(inputs[k], dtype=np.float32) for k in
          ("ln_g", "ln_b", "proj_w", "proj_b", "w1", "b1", "w2", "b2", "w3", "b3"))
    )
    bounds = np.searchsorted(batch, np.arange(B + 1))
    return [
        {"xt": pack_feat_core(feat16, feat32, bounds, c), **wdict}
        for c in range(N_CORES)
    ]


def kernel(**inputs):
    from concourse.bass_utils import run_bass_kernel_spmd

    if "nc" not in _CACHE:
        _CACHE["nc"] = build_nc()
    nc = _CACHE["nc"]

    in_maps = make_in_maps(inputs)
    res = run_bass_kernel_spmd(nc, in_maps, list(range(N_CORES)))

    out = np.empty((B, OUT_F), np.float32)
    for c in range(N_CORES):
        out[c * 16 : (c + 1) * 16] = res.results[c]["out"].astype(np.float32)
    out += np.asarray(inputs["b3"], dtype=np.float32)[None, :]
    return out.reshape(B, 2048, 3)


# revision 10
# speedup vs baseline: 1.3255x; 1.0987x over previous
import numpy as np

B = 128
FEAT = 64
LATENT = 512
OUT_F = 6144  # NUM_POINTS * 3
EPS = 1e-5
N_CORES = 8
SEGS_PER_CORE = 16
S_PAD = 8192
F16MIN = np.float16(-65504.0)

_CACHE = {}


def build_nc():
    from concourse import bass, bacc, tile

    mybir = bass.mybir
    f32 = mybir.dt.float32
    f16 = mybir.dt.float16
    bf16 = mybir.dt.bfloat16
    AF = mybir.ActivationFunctionType
    ALU = mybir.AluOpType
    X = mybir.AxisListType.X

    nc = bacc.Bacc("TRN2")
    xt_d = nc.declare_dram_parameter("xt", [128, 8, S_PAD], f16, isOutput=False)
    wp_d = nc.declare_dram_parameter("wp2", [128, LATENT], bf16, isOutput=False)
    bp_d = nc.declare_dram_parameter("bp", [128, 4], f32, isOutput=False)
    w1_d = nc.declare_dram_parameter("w1p", [128, 1024], bf16, isOutput=False)
    b1_d = nc.declare_dram_parameter("b1p", [128, 2], f32, isOutput=False)
    w2_d = nc.declare_dram_parameter("w2p", [128, 1024], bf16, isOutput=False)
    b2_d = nc.declare_dram_parameter("b2p", [128, 4], f32, isOutput=False)
    w3_d = nc.declare_dram_parameter("w3p", [128, 3, 4, 2048], bf16, isOutput=False)
    selT_d = nc.declare_dram_parameter("selT", [128, 2], f32, isOutput=False)
    sel_d = nc.declare_dram_parameter("sel", [2, 128], f32, isOutput=False)
    # col-tiled GEMM output: row 32*sub+s, col 512*g+n -> out[16c+s, 2048g+512sub+n]
    out_d = nc.declare_dram_parameter("out", [128, 1536], f16, isOutput=True)

    with tile.TileContext(nc) as tc:
        with (
            tc.tile_pool(name="wpool", bufs=1) as wpool,
            tc.tile_pool(name="fpool", bufs=6) as fpool,
            tc.tile_pool(name="spool", bufs=1) as spool,
            tc.tile_pool(name="ps_s", bufs=1, space=bass.MemorySpace.PSUM) as ps_s,
            tc.tile_pool(name="ps_b", bufs=1, space=bass.MemorySpace.PSUM) as ps_b,
            tc.tile_pool(name="ps_m", bufs=3, space=bass.MemorySpace.PSUM) as ps_m,
            tc.tile_pool(name="ps_o", bufs=2, space=bass.MemorySpace.PSUM) as ps_o,
        ):
            wp = wpool.tile([128, LATENT], bf16)
            bp = wpool.tile([128, 4], f32)
            w1 = wpool.tile([128, 1024], bf16)
            b1 = wpool.tile([128, 2], f32)
            w2 = wpool.tile([128, 1024], bf16)
            b2 = wpool.tile([128, 4], f32)
            sel2T = wpool.tile([128, 2], f32)
            sel2 = wpool.tile([2, 128], f32)
            eps_t = wpool.tile([2, 1], f32)
            scr = wpool.tile([2, 1], f32)
            w3t = wpool.tile([128, 3, 4, 2048], bf16)

            nc.gpsimd.dma_start(sel2T[:], selT_d[:])
            nc.gpsimd.dma_start(sel2[:], sel_d[:])
            for t, d in (
                (wp, wp_d), (bp, bp_d), (w1, w1_d), (b1, b1_d),
                (w2, w2_d), (b2, b2_d),
            ):
                nc.gpsimd.dma_start(t[:], d[:])

            nc.vector.memset(eps_t[:], EPS)
            # hoist the Sqrt activation-table load out of the tail
            nc.scalar.activation(scr[:], eps_t[:], AF.Sqrt)

            val_h = spool.tile([128, 8], f16)
            val32 = spool.tile([128, 8], f32)
            mu2 = spool.tile([2, 8], f32)
            std = spool.tile([2, 8], f32)
            rstd = spool.tile([2, 8], f32)
            zc = spool.tile([128, 8], f32)
            zsq = spool.tile([128, 8], f32)
            zn16 = spool.tile([128, 16], bf16)
            lat = spool.tile([128, 64], bf16)
            h1 = spool.tile([128, 32], bf16)
            h2 = spool.tile([128, 64], bf16)
            out_sb = spool.tile([128, 1536], f16)
            nc.vector.memset(zn16[:], 0.0)

            # --- segment max pooling. Per tile [128, 8, 1024]: 7 in-place
            # tensor_tensor max folds (2-byte TT runs 2 elem/cyc on DVE vs
            # reduce's 1) then a short 1024-col reduce. ---
            def fold_reduce(ft, t):
                for j in range(1, 8):
                    nc.vector.tensor_tensor(
                        ft[:, 0, :], ft[:, 0, :], ft[:, j, :], op=ALU.max
                    )
                nc.vector.reduce_max(val_h[:, t : t + 1], ft[:, 0, :], axis=X)

            for t in range(6):
                ft = fpool.tile([128, 8, 1024], f16, name="ft")
                nc.sync.dma_start(ft[:], xt_d[:, t, :])
                fold_reduce(ft, t)

            # slots 6,7: 4 chunked DMAs of 2048 cols; fold chunks 1..3 into
            # chunk 0's region as they arrive, then reduce chunk 0
            cht = {}
            for t in (6, 7):
                cht[t] = fpool.tile([128, 4, 2048], f16, name="ft")
            for ci in range(4):
                for t in (6, 7):
                    nc.sync.dma_start(
                        cht[t][:, ci, :], xt_d[:, t, 2048 * ci : 2048 * (ci + 1)]
                    )
                    if ci > 0:
                        nc.vector.tensor_tensor(
                            cht[t][:, 0, :], cht[t][:, 0, :], cht[t][:, ci, :],
                            op=ALU.max,
                        )
            for t in (6, 7):
                nc.vector.reduce_max(val_h[:, t : t + 1], cht[t][:, 0, :], axis=X)
            nc.vector.tensor_copy(val32[:], val_h[:])

            # w3 is only needed for the tail GEMM: stream it on the HWDGE
            # queues BEHIND all feat tiles so it never competes with them
            # (on the SWDGE queue its large packets win ~70% of the
            # engine round-robin and starve the feat stream)
            nc.scalar.dma_start(w3t[:], w3_d[:])

            # --- PE warm-up burst, gated on slot 5's pooled value (lands a
            # few us before stream end) so HAM is warm for the tail matmuls ---
            gate = ps_s.tile([2, 1], f32, name="gate")
            nc.tensor.matmul(
                gate[:], val_h[:, 0:2], val_h[:, 5:6], start=True, stop=True
            )
            wps = ps_o.tile([128, 512], f32, name="pso")
            for _ in range(12):
                nc.tensor.matmul(
                    wps[0:16, :], w1[:, 0:16], w1[:, 0:512],
                    start=True, stop=True,
                )

            # --- LayerNorm per (group, col) on val32 [128, 8] ---
            red = ps_s.tile([2, 8], f32, name="red")
            nc.tensor.matmul(red[:], sel2T[:], val32[:], start=True, stop=True)
            nc.scalar.mul(mu2[:], red[:], 1.0 / FEAT)
            bc = ps_b.tile([128, 8], f32, name="bc")
            nc.tensor.matmul(bc[:], sel2[:], mu2[:], start=True, stop=True)
            nc.vector.tensor_tensor(zc[:], val32[:], bc[:], op=ALU.subtract)
            nc.vector.tensor_tensor(zsq[:], zc[:], zc[:], op=ALU.mult)
            red2 = ps_s.tile([2, 8], f32, name="red")
            nc.tensor.matmul(red2[:], sel2T[:], zsq[:], start=True, stop=True)
            nc.scalar.activation(
                std[:], red2[:], AF.Sqrt, bias=eps_t[:], scale=1.0 / FEAT
            )
            nc.vector.reciprocal(rstd[:], std[:])
            bc2 = ps_b.tile([128, 8], f32, name="bc")
            nc.tensor.matmul(bc2[:], sel2[:], rstd[:], start=True, stop=True)
            nc.vector.tensor_tensor(
                zn16[0:64, 0:8], zc[0:64, :], bc2[0:64, :], op=ALU.mult
            )
            nc.vector.tensor_tensor(
                zn16[64:128, 8:16], zc[64:128, :], bc2[64:128, :], op=ALU.mult
            )

            # --- proj (ln affine folded into wp/bp): lat[128m+p, s] ---
            for m in range(4):
                ps = ps_m.tile([128, 16], f32)
                nc.tensor.matmul(
                    ps[:], wp[:, 128 * m : 128 * (m + 1)], zn16[:],
                    start=True, stop=True,
                )
                nc.vector.tensor_scalar(
                    lat[:, 16 * m : 16 * (m + 1)], ps[:], bp[:, m : m + 1],
                    None, op0=ALU.add,
                )

            # --- h1 = relu(latent @ w1 + b1), transposed ---
            for n in range(2):
                ps = ps_m.tile([128, 16], f32)
                for k in range(4):
                    nc.tensor.matmul(
                        ps[:],
                        w1[:, (k * 2 + n) * 128 : (k * 2 + n + 1) * 128],
                        lat[:, 16 * k : 16 * (k + 1)],
                        start=(k == 0), stop=(k == 3),
                    )
                nc.vector.tensor_scalar(
                    h1[:, 16 * n : 16 * (n + 1)], ps[:], b1[:, n : n + 1],
                    0.0, op0=ALU.add, op1=ALU.max,
                )

            # --- h2 = relu(h1 @ w2 + b2), transposed ---
            for n in range(4):
                ps = ps_m.tile([128, 16], f32)
                for k in range(2):
                    nc.tensor.matmul(
                        ps[:],
                        w2[:, (k * 4 + n) * 128 : (k * 4 + n + 1) * 128],
                        h1[:, 16 * k : 16 * (k + 1)],
                        start=(k == 0), stop=(k == 1),
                    )
                nc.vector.tensor_scalar(
                    h2[:, 16 * n : 16 * (n + 1)], ps[:], b2[:, n : n + 1],
                    0.0, op0=ALU.add, op1=ALU.max,
                )

            # --- out = h2.T @ w3, col-tiled: 4 concurrent M=16 matmuls per
            # PSUM bank (tile_position col groups), copies split across
            # Vector/Scalar engines ---
            for g in range(3):
                psg = ps_o.tile([128, 512], f32, name="pso")
                for sub in range(4):
                    for k in range(4):
                        nc.tensor.matmul(
                            psg[32 * sub : 32 * sub + 16, :],
                            h2[:, 16 * k : 16 * (k + 1)],
                            w3t[:, g, k, 512 * sub : 512 * (sub + 1)],
                            start=(k == 0), stop=(k == 3),
                            tile_position=(0, 32 * sub),
                        )
                for sub in range(4):
                    src = psg[32 * sub : 32 * sub + 16, :]
                    dst = out_sb[32 * sub : 32 * sub + 16, 512 * g : 512 * (g + 1)]
                    if sub % 2 == 0:
                        nc.vector.tensor_copy(dst, src)
                    else:
                        nc.scalar.copy(dst, src)
                nc.sync.dma_start(
                    out_d[:, 512 * g : 512 * (g + 1)],
                    out_sb[:, 512 * g : 512 * (g + 1)],
                )

    nc.finalize()
    return nc


def _bf16(a):
    import ml_dtypes

    return np.ascontiguousarray(a.astype(ml_dtypes.bfloat16))


def pack_weights(ln_g, ln_b, proj_w, proj_b, w1, b1, w2, b2, w3, b3):
    c = np.ascontiguousarray
    wp = (ln_g[:, None] * proj_w).astype(np.float32)  # [64, 512]
    bpv = (ln_b.astype(np.float64) @ proj_w.astype(np.float64)).astype(np.float32) + proj_b
    return {
        "wp2": _bf16(np.vstack([wp, wp])),
        "bp": c(bpv.reshape(4, 128).T),
        "w1p": _bf16(
            w1.reshape(4, 128, 2, 128).transpose(1, 0, 2, 3).reshape(128, 1024)
        ),
        "b1p": c(b1.reshape(2, 128).T),
        "w2p": _bf16(
            w2.reshape(2, 128, 4, 128).transpose(1, 0, 2, 3).reshape(128, 1024)
        ),
        "b2p": c(b2.reshape(4, 128).T),
        "w3p": _bf16(w3.reshape(4, 128, 3, 2048).transpose(1, 2, 0, 3)),
        "selT": c(np.repeat(np.eye(2, dtype=np.float32), 64, axis=0)),
        "sel": c(np.repeat(np.eye(2, dtype=np.float32), 64, axis=1)),
    }


def pack_feat_core(feat16, feat32, bounds, c):
    xt = np.full((128, 8, S_PAD), F16MIN, np.float16)
    for sl in range(SEGS_PER_CORE):
        seg = c * SEGS_PER_CORE + sl
        a, b = bounds[seg], bounds[seg + 1]
        L = b - a
        if L > S_PAD:
            blk = np.concatenate(
                [
                    feat16[a : a + S_PAD - 1],
                    feat32[a + S_PAD - 1 : b].max(0, keepdims=True).astype(np.float16),
                ],
                0,
            )
            L = S_PAD
        else:
            blk = feat16[a:b]
        g, t = divmod(sl, 8)
        if L > 0:
            xt[g * 64 : (g + 1) * 64, t, :L] = blk.T
    return xt


def make_in_maps(inputs):
    feat32 = np.asarray(inputs["feat"], dtype=np.float32)
    feat16 = feat32.astype(np.float16)
    batch = np.asarray(inputs["batch"])
    wdict = pack_weights(
        *(np.asarray(inputs[k], dtype=np.float32) for k in
          ("ln_g", "ln_b", "proj_w", "proj_b", "w1", "b1", "w2", "b2", "w3", "b3"))
    )
    bounds = np.searchsorted(batch, np.arange(B + 1))
    return [
        {"xt": pack_feat_core(feat16, feat32, bounds, c), **wdict}
        for c in range(N_CORES)
    ]


def kernel(**inputs):
    from concourse.bass_utils import run_bass_kernel_spmd

    if "nc" not in _CACHE:
        _CACHE["nc"] = build_nc()
    nc = _CACHE["nc"]

    in_maps = make_in_maps(inputs)
    res = run_bass_kernel_spmd(nc, in_maps, list(range(N_CORES)))

    out = np.empty((B, OUT_F), np.float32)
    for c in range(N_CORES):
        r = np.asarray(res.results[c]["out"], dtype=np.float32)  # [128, 1536]
        # row 32*sub+s, col 512*g+n -> out[16c+s, 2048g+512sub+n]
        blk = r.reshape(4, 32, 3, 512)[:, :16]  # [sub, s, g, n]
        out[c * 16 : (c + 1) * 16] = blk.transpose(1, 2, 0, 3).reshape(16, OUT_F)
    out += np.asarray(inputs["b3"], dtype=np.float32)[None, :]
    return out.reshape(B, 2048, 3)


# revision 11
# speedup vs baseline: 1.3391x; 1.0102x over previous
import numpy as np

B = 128
FEAT = 64
LATENT = 512
OUT_F = 6144  # NUM_POINTS * 3
EPS = 1e-5
N_CORES = 8
SEGS_PER_CORE = 16
S_PAD = 8192
F16MIN = np.float16(-65504.0)

_CACHE = {}


def build_nc():
    from concourse import bass, bacc, tile

    mybir = bass.mybir
    f32 = mybir.dt.float32
    f16 = mybir.dt.float16
    bf16 = mybir.dt.bfloat16
    AF = mybir.ActivationFunctionType
    ALU = mybir.AluOpType
    X = mybir.AxisListType.X

    nc = bacc.Bacc("TRN2")
    xt_d = nc.declare_dram_parameter("xt", [128, 8, S_PAD], f16, isOutput=False)
    wp_d = nc.declare_dram_parameter("wp2", [128, LATENT], bf16, isOutput=False)
    bp_d = nc.declare_dram_parameter("bp", [128, 4], f32, isOutput=False)
    w1_d = nc.declare_dram_parameter("w1p", [128, 1024], bf16, isOutput=False)
    b1_d = nc.declare_dram_parameter("b1p", [128, 2], f32, isOutput=False)
    w2_d = nc.declare_dram_parameter("w2p", [128, 1024], bf16, isOutput=False)
    b2_d = nc.declare_dram_parameter("b2p", [128, 4], f32, isOutput=False)
    w3_d = nc.declare_dram_parameter("w3p", [128, 3, 4, 2048], bf16, isOutput=False)
    selT_d = nc.declare_dram_parameter("selT", [128, 2], f32, isOutput=False)
    sel_d = nc.declare_dram_parameter("sel", [2, 128], f32, isOutput=False)
    # col-tiled GEMM output: row 32*sub+s, col 512*g+n -> out[16c+s, 2048g+512sub+n]
    out_d = nc.declare_dram_parameter("out", [128, 1536], f16, isOutput=True)

    with tile.TileContext(nc) as tc:
        with (
            tc.tile_pool(name="wpool", bufs=1) as wpool,
            tc.tile_pool(name="fpool", bufs=6) as fpool,
            tc.tile_pool(name="spool", bufs=1) as spool,
            tc.tile_pool(name="ps_s", bufs=1, space=bass.MemorySpace.PSUM) as ps_s,
            tc.tile_pool(name="ps_b", bufs=1, space=bass.MemorySpace.PSUM) as ps_b,
            tc.tile_pool(name="ps_m", bufs=3, space=bass.MemorySpace.PSUM) as ps_m,
            tc.tile_pool(name="ps_o", bufs=2, space=bass.MemorySpace.PSUM) as ps_o,
        ):
            wp = wpool.tile([128, LATENT], bf16)
            bp = wpool.tile([128, 4], f32)
            w1 = wpool.tile([128, 1024], bf16)
            b1 = wpool.tile([128, 2], f32)
            w2 = wpool.tile([128, 1024], bf16)
            b2 = wpool.tile([128, 4], f32)
            sel2T = wpool.tile([128, 2], f32)
            sel2 = wpool.tile([2, 128], f32)
            eps_t = wpool.tile([2, 1], f32)
            scr = wpool.tile([2, 1], f32)
            scr2 = wpool.tile([128, 3], f16)
            w3t = wpool.tile([128, 3, 4, 2048], bf16)

            nc.gpsimd.dma_start(sel2T[:], selT_d[:])
            nc.gpsimd.dma_start(sel2[:], sel_d[:])
            for t, d in (
                (wp, wp_d), (bp, bp_d), (w1, w1_d), (b1, b1_d),
                (w2, w2_d), (b2, b2_d),
            ):
                nc.gpsimd.dma_start(t[:], d[:])

            nc.vector.memset(eps_t[:], EPS)
            # hoist the Sqrt activation-table load out of the tail
            nc.scalar.activation(scr[:], eps_t[:], AF.Sqrt)

            val_h = spool.tile([128, 8], f16)
            val32 = spool.tile([128, 8], f32)
            mu2 = spool.tile([2, 8], f32)
            std = spool.tile([2, 8], f32)
            rstd = spool.tile([2, 8], f32)
            zc = spool.tile([128, 8], f32)
            zsq = spool.tile([128, 8], f32)
            zn16 = spool.tile([128, 16], bf16)
            lat = spool.tile([128, 64], bf16)
            h1 = spool.tile([128, 32], bf16)
            h2 = spool.tile([128, 64], bf16)
            out_sb = spool.tile([128, 1536], f16)
            nc.vector.memset(zn16[:], 0.0)

            # --- segment max pooling. Per tile [128, 8, 1024]: 7 in-place
            # tensor_tensor max folds (2-byte TT runs 2 elem/cyc on DVE vs
            # reduce's 1) then a short 1024-col reduce. ---
            def fold_reduce(ft, t):
                for j in range(1, 8):
                    nc.vector.tensor_tensor(
                        ft[:, 0, :], ft[:, 0, :], ft[:, j, :], op=ALU.max
                    )
                nc.vector.reduce_max(val_h[:, t : t + 1], ft[:, 0, :], axis=X)

            for t in range(6):
                ft = fpool.tile([128, 8, 1024], f16, name="ft")
                nc.sync.dma_start(ft[:], xt_d[:, t, :])
                fold_reduce(ft, t)

            # slots 6,7: 4 chunked DMAs of 2048 cols; fold chunks 1..3 into
            # chunk 0's region as they arrive, then reduce chunk 0
            cht = {}
            for t in (6, 7):
                cht[t] = fpool.tile([128, 4, 2048], f16, name="ft")
            for ci in range(4):
                for t in (6, 7):
                    nc.sync.dma_start(
                        cht[t][:, ci, :], xt_d[:, t, 2048 * ci : 2048 * (ci + 1)]
                    )
                    if ci > 0:
                        nc.vector.tensor_tensor(
                            cht[t][:, 0, :], cht[t][:, 0, :], cht[t][:, ci, :],
                            op=ALU.max,
                        )
            for t in (6, 7):
                nc.vector.reduce_max(val_h[:, t : t + 1], cht[t][:, 0, :], axis=X)
            nc.vector.tensor_copy(val32[:], val_h[:])

            # w3 is only needed for the tail GEMM. Its 48KB-contiguous
            # descriptors produce ~2us packets that win the per-packet
            # engine round-robin and starve the feat stream, so trickle it
            # in three 1MB chunks, each gated behind feat-stream progress
            # via a scalar-engine copy (the ACT FIFO delays the triggers).
            for g in range(3):
                nc.scalar.copy(scr2[:, g : g + 1], val_h[:, 2 * g + 1 : 2 * g + 2])
                nc.scalar.dma_start(w3t[:, g, :, :], w3_d[:, g, :, :])

            # --- PE warm-up burst, gated on slot 5's pooled value (lands a
            # few us before stream end) so HAM is warm for the tail matmuls ---
            gate = ps_s.tile([2, 1], f32, name="gate")
            nc.tensor.matmul(
                gate[:], val_h[:, 0:2], val_h[:, 5:6], start=True, stop=True
            )
            wps = ps_o.tile([128, 512], f32, name="pso")
            for _ in range(12):
                nc.tensor.matmul(
                    wps[0:16, :], w1[:, 0:16], w1[:, 0:512],
                    start=True, stop=True,
                )

            # --- LayerNorm per (group, col) on val32 [128, 8] ---
            red = ps_s.tile([2, 8], f32, name="red")
            nc.tensor.matmul(red[:], sel2T[:], val32[:], start=True, stop=True)
            nc.scalar.mul(mu2[:], red[:], 1.0 / FEAT)
            bc = ps_b.tile([128, 8], f32, name="bc")
            nc.tensor.matmul(bc[:], sel2[:], mu2[:], start=True, stop=True)
            nc.vector.tensor_tensor(zc[:], val32[:], bc[:], op=ALU.subtract)
            nc.vector.tensor_tensor(zsq[:], zc[:], zc[:], op=ALU.mult)
            red2 = ps_s.tile([2, 8], f32, name="red")
            nc.tensor.matmul(red2[:], sel2T[:], zsq[:], start=True, stop=True)
            nc.scalar.activation(
                std[:], red2[:], AF.Sqrt, bias=eps_t[:], scale=1.0 / FEAT
            )
            nc.vector.reciprocal(rstd[:], std[:])
            bc2 = ps_b.tile([128, 8], f32, name="bc")
            nc.tensor.matmul(bc2[:], sel2[:], rstd[:], start=True, stop=True)
            nc.vector.tensor_tensor(
                zn16[0:64, 0:8], zc[0:64, :], bc2[0:64, :], op=ALU.mult
            )
            nc.vector.tensor_tensor(
                zn16[64:128, 8:16], zc[64:128, :], bc2[64:128, :], op=ALU.mult
            )

            # --- proj (ln affine folded into wp/bp): lat[128m+p, s] ---
            for m in range(4):
                ps = ps_m.tile([128, 16], f32)
                nc.tensor.matmul(
                    ps[:], wp[:, 128 * m : 128 * (m + 1)], zn16[:],
                    start=True, stop=True,
                )
                nc.vector.tensor_scalar(
                    lat[:, 16 * m : 16 * (m + 1)], ps[:], bp[:, m : m + 1],
                    None, op0=ALU.add,
                )

            # --- h1 = relu(latent @ w1 + b1), transposed ---
            for n in range(2):
                ps = ps_m.tile([128, 16], f32)
                for k in range(4):
                    nc.tensor.matmul(
                        ps[:],
                        w1[:, (k * 2 + n) * 128 : (k * 2 + n + 1) * 128],
                        lat[:, 16 * k : 16 * (k + 1)],
                        start=(k == 0), stop=(k == 3),
                    )
                nc.vector.tensor_scalar(
                    h1[:, 16 * n : 16 * (n + 1)], ps[:], b1[:, n : n + 1],
                    0.0, op0=ALU.add, op1=ALU.max,
                )

            # --- h2 = relu(h1 @ w2 + b2), transposed ---
            for n in range(4):
                ps = ps_m.tile([128, 16], f32)
                for k in range(2):
                    nc.tensor.matmul(
                        ps[:],
                        w2[:, (k * 4 + n) * 128 : (k * 4 + n + 1) * 128],
                        h1[:, 16 * k : 16 * (k + 1)],
                        start=(k == 0), stop=(k == 1),
                    )
                nc.vector.tensor_scalar(
                    h2[:, 16 * n : 16 * (n + 1)], ps[:], b2[:, n : n + 1],
                    0.0, op0=ALU.add, op1=ALU.max,
                )

            # --- out = h2.T @ w3, col-tiled: 4 concurrent M=16 matmuls per
            # PSUM bank (tile_position col groups), copies split across
            # Vector/Scalar engines ---
            for g in range(3):
                psg = ps_o.tile([128, 512], f32, name="pso")
                for sub in range(4):
                    for k in range(4):
                        nc.tensor.matmul(
                            psg[32 * sub : 32 * sub + 16, :],
                            h2[:, 16 * k : 16 * (k + 1)],
                            w3t[:, g, k, 512 * sub : 512 * (sub + 1)],
                            start=(k == 0), stop=(k == 3),
                            tile_position=(0, 32 * sub),
                        )
                for sub in range(4):
                    src = psg[32 * sub : 32 * sub + 16, :]
                    dst = out_sb[32 * sub : 32 * sub + 16, 512 * g : 512 * (g + 1)]
                    if sub % 2 == 0:
                        nc.vector.tensor_copy(dst, src)
                    else:
                        nc.scalar.copy(dst, src)
                nc.sync.dma_start(
                    out_d[:, 512 * g : 512 * (g + 1)],
                    out_sb[:, 512 * g : 512 * (g + 1)],
                )

    nc.finalize()
    return nc


def _bf16(a):
    import ml_dtypes

    return np.ascontiguousarray(a.astype(ml_dtypes.bfloat16))


def pack_weights(ln_g, ln_b, proj_w, proj_b, w1, b1, w2, b2, w3, b3):
    c = np.ascontiguousarray
    wp = (ln_g[:, None] * proj_w).astype(np.float32)  # [64, 512]
    bpv = (ln_b.astype(np.float64) @ proj_w.astype(np.float64)).astype(np.float32) + proj_b
    return {
        "wp2": _bf16(np.vstack([wp, wp])),
        "bp": c(bpv.reshape(4, 128).T),
        "w1p": _bf16(
            w1.reshape(4, 128, 2, 128).transpose(1, 0, 2, 3).reshape(128, 1024)
        ),
        "b1p": c(b1.reshape(2, 128).T),
        "w2p": _bf16(
            w2.reshape(2, 128, 4, 128).transpose(1, 0, 2, 3).reshape(128, 1024)
        ),
        "b2p": c(b2.reshape(4, 128).T),
        "w3p": _bf16(w3.reshape(4, 128, 3, 2048).transpose(1, 2, 0, 3)),
        "selT": c(np.repeat(np.eye(2, dtype=np.float32), 64, axis=0)),
        "sel": c(np.repeat(np.eye(2, dtype=np.float32), 64, axis=1)),
    }


def pack_feat_core(feat16, feat32, bounds, c):
    xt = np.full((128, 8, S_PAD), F16MIN, np.float16)
    for sl in range(SEGS_PER_CORE):
        seg = c * SEGS_PER_CORE + sl
        a, b = bounds[seg], bounds[seg + 1]
        L = b - a
        if L > S_PAD:
            blk = np.concatenate(
                [
                    feat16[a : a + S_PAD - 1],
                    feat32[a + S_PAD - 1 : b].max(0, keepdims=True).astype(np.float16),
                ],
                0,
            )
            L = S_PAD
        else:
            blk = feat16[a:b]
        g, t = divmod(sl, 8)
        if L > 0:
            xt[g * 64 : (g + 1) * 64, t, :L] = blk.T
    return xt


def make_in_maps(inputs):
    feat32 = np.asarray(inputs["feat"], dtype=np.float32)
    feat16 = feat32.astype(np.float16)
    batch = np.asarray(inputs["batch"])
    wdict = pack_weights(
        *(np.asarray(inputs[k], dtype=np.float32) for k in
          ("ln_g", "ln_b", "proj_w", "proj_b", "w1", "b1", "w2", "b2", "w3", "b3"))
    )
    bounds = np.searchsorted(batch, np.arange(B + 1))
    return [
        {"xt": pack_feat_core(feat16, feat32, bounds, c), **wdict}
        for c in range(N_CORES)
    ]


def kernel(**inputs):
    from concourse.bass_utils import run_bass_kernel_spmd

    if "nc" not in _CACHE:
        _CACHE["nc"] = build_nc()
    nc = _CACHE["nc"]

    in_maps = make_in_maps(inputs)
    res = run_bass_kernel_spmd(nc, in_maps, list(range(N_CORES)))

    out = np.empty((B, OUT_F), np.float32)
    for c in range(N_CORES):
        r = np.asarray(res.results[c]["out"], dtype=np.float32)  # [128, 1536]
        # row 32*sub+s, col 512*g+n -> out[16c+s, 2048g+512sub+n]
        blk = r.reshape(4, 32, 3, 512)[:, :16]  # [sub, s, g, n]
        out[c * 16 : (c + 1) * 16] = blk.transpose(1, 2, 0, 3).reshape(16, OUT_F)
    out += np.asarray(inputs["b3"], dtype=np.float32)[None, :]
    return out.reshape(B, 2048, 3)


# revision 12
# speedup vs baseline: 1.4456x; 1.0795x over previous
import numpy as np

B = 128
FEAT = 64
LATENT = 512
OUT_F = 6144  # NUM_POINTS * 3
EPS = 1e-5
N_CORES = 8
SEGS_PER_CORE = 16
S_PAD = 8192
F16MIN = np.float16(-65504.0)

_CACHE = {}


def build_nc():
    from concourse import bass, bacc, tile

    mybir = bass.mybir
    f32 = mybir.dt.float32
    f16 = mybir.dt.float16
    bf16 = mybir.dt.bfloat16
    AF = mybir.ActivationFunctionType
    ALU = mybir.AluOpType
    X = mybir.AxisListType.X

    nc = bacc.Bacc("TRN2")
    xt_d = nc.declare_dram_parameter("xt", [128, 8, S_PAD], f16, isOutput=False)
    wp_d = nc.declare_dram_parameter("wp2", [128, LATENT], bf16, isOutput=False)
    bp_d = nc.declare_dram_parameter("bp", [128, 4], f32, isOutput=False)
    w1_d = nc.declare_dram_parameter("w1p", [128, 1024], bf16, isOutput=False)
    b1_d = nc.declare_dram_parameter("b1p", [128, 2], f32, isOutput=False)
    w2_d = nc.declare_dram_parameter("w2p", [128, 1024], bf16, isOutput=False)
    b2_d = nc.declare_dram_parameter("b2p", [128, 4], f32, isOutput=False)
    w3_d = nc.declare_dram_parameter("w3p", [128, 3, 4, 2048], bf16, isOutput=False)
    selT_d = nc.declare_dram_parameter("selT", [128, 2], f32, isOutput=False)
    sel_d = nc.declare_dram_parameter("sel", [2, 128], f32, isOutput=False)
    # col-tiled GEMM output: row 32*sub+s, col 512*g+n -> out[16c+s, 2048g+512sub+n]
    out_d = nc.declare_dram_parameter("out", [128, 1536], f16, isOutput=True)

    with tile.TileContext(nc) as tc:
        with (
            tc.tile_pool(name="wpool", bufs=1) as wpool,
            tc.tile_pool(name="fpool", bufs=6) as fpool,
            tc.tile_pool(name="spool", bufs=1) as spool,
            tc.tile_pool(name="ps_s", bufs=1, space=bass.MemorySpace.PSUM) as ps_s,
            tc.tile_pool(name="ps_b", bufs=1, space=bass.MemorySpace.PSUM) as ps_b,
            tc.tile_pool(name="ps_m", bufs=3, space=bass.MemorySpace.PSUM) as ps_m,
            tc.tile_pool(name="ps_o", bufs=2, space=bass.MemorySpace.PSUM) as ps_o,
        ):
            wp = wpool.tile([128, LATENT], bf16)
            bp = wpool.tile([128, 4], f32)
            w1 = wpool.tile([128, 1024], bf16)
            b1 = wpool.tile([128, 2], f32)
            w2 = wpool.tile([128, 1024], bf16)
            b2 = wpool.tile([128, 4], f32)
            sel2T = wpool.tile([128, 2], f32)
            sel2 = wpool.tile([2, 128], f32)
            eps_t = wpool.tile([2, 1], f32)
            scr = wpool.tile([2, 1], f32)
            scr2 = wpool.tile([128, 3], f16)
            w3t = wpool.tile([128, 3, 4, 2048], bf16)

            nc.gpsimd.dma_start(sel2T[:], selT_d[:])
            nc.gpsimd.dma_start(sel2[:], sel_d[:])
            for t, d in (
                (wp, wp_d), (bp, bp_d), (w1, w1_d), (b1, b1_d),
                (w2, w2_d), (b2, b2_d),
            ):
                nc.gpsimd.dma_start(t[:], d[:])

            nc.vector.memset(eps_t[:], EPS)
            # hoist the Sqrt activation-table load out of the tail
            nc.scalar.activation(scr[:], eps_t[:], AF.Sqrt)

            val_h = spool.tile([128, 8], f16)
            val32 = spool.tile([128, 8], f32)
            mu2 = spool.tile([2, 8], f32)
            std = spool.tile([2, 8], f32)
            rstd = spool.tile([2, 8], f32)
            zc = spool.tile([128, 8], f32)
            zsq = spool.tile([128, 8], f32)
            zn16 = spool.tile([128, 16], bf16)
            lat = spool.tile([128, 64], bf16)
            h1 = spool.tile([128, 32], bf16)
            h2 = spool.tile([128, 64], bf16)
            out_sb = spool.tile([128, 1536], f16)
            nc.vector.memset(zn16[:], 0.0)

            # --- segment max pooling. Per tile [128, 8, 1024]: 7 in-place
            # tensor_tensor max folds (2-byte TT runs 2 elem/cyc on DVE vs
            # reduce's 1) then a short 1024-col reduce. ---
            def fold_reduce(ft, t):
                for j in range(1, 8):
                    nc.vector.tensor_tensor(
                        ft[:, 0, :], ft[:, 0, :], ft[:, j, :], op=ALU.max
                    )
                nc.vector.reduce_max(val_h[:, t : t + 1], ft[:, 0, :], axis=X)

            for t in range(6):
                ft = fpool.tile([128, 8, 1024], f16, name="ft")
                nc.sync.dma_start(ft[:], xt_d[:, t, :])
                fold_reduce(ft, t)

            # slots 6,7: 4 chunked DMAs of 2048 cols; fold chunks 1..3 into
            # chunk 0's region as they arrive, then reduce chunk 0
            cht = {}
            for t in (6, 7):
                cht[t] = fpool.tile([128, 4, 2048], f16, name="ft")
            for ci in range(4):
                for t in (6, 7):
                    nc.sync.dma_start(
                        cht[t][:, ci, :], xt_d[:, t, 2048 * ci : 2048 * (ci + 1)]
                    )
                    if ci > 0:
                        nc.vector.tensor_tensor(
                            cht[t][:, 0, :], cht[t][:, 0, :], cht[t][:, ci, :],
                            op=ALU.max,
                        )
            for t in (6, 7):
                nc.vector.reduce_max(val_h[:, t : t + 1], cht[t][:, 0, :], axis=X)
            nc.vector.tensor_copy(val32[:], val_h[:])

            # w3 is only needed for the tail GEMM. Its 48KB-contiguous
            # descriptors produce ~2us packets that win the per-packet
            # engine round-robin and starve the feat stream, so trickle it
            # in three 1MB chunks, each gated behind feat-stream progress
            # via a scalar-engine copy (the ACT FIFO delays the triggers).
            for g in range(3):
                nc.scalar.copy(scr2[:, g : g + 1], val_h[:, 2 * g + 1 : 2 * g + 2])
                nc.scalar.dma_start(w3t[:, g, :, :], w3_d[:, g, :, :])

            # --- PE warm-up burst, gated on slot 5's pooled value (lands a
            # few us before stream end) so HAM is warm for the tail matmuls ---
            gate = ps_s.tile([2, 1], f32, name="gate")
            nc.tensor.matmul(
                gate[:], val_h[:, 0:2], val_h[:, 5:6], start=True, stop=True
            )
            wps = ps_o.tile([128, 512], f32, name="pso")
            for _ in range(12):
                nc.tensor.matmul(
                    wps[0:16, :], w1[:, 0:16], w1[:, 0:512],
                    start=True, stop=True,
                )

            # --- LayerNorm per (group, col) on val32 [128, 8] ---
            red = ps_s.tile([2, 8], f32, name="red")
            nc.tensor.matmul(red[:], sel2T[:], val32[:], start=True, stop=True)
            nc.scalar.mul(mu2[:], red[:], 1.0 / FEAT)
            bc = ps_b.tile([128, 8], f32, name="bc")
            nc.tensor.matmul(bc[:], sel2[:], mu2[:], start=True, stop=True)
            nc.vector.tensor_tensor(zc[:], val32[:], bc[:], op=ALU.subtract)
            nc.vector.tensor_tensor(zsq[:], zc[:], zc[:], op=ALU.mult)
            red2 = ps_s.tile([2, 8], f32, name="red")
            nc.tensor.matmul(red2[:], sel2T[:], zsq[:], start=True, stop=True)
            nc.scalar.activation(
                std[:], red2[:], AF.Sqrt, bias=eps_t[:], scale=1.0 / FEAT
            )
            nc.vector.reciprocal(rstd[:], std[:])
            bc2 = ps_b.tile([128, 8], f32, name="bc")
            nc.tensor.matmul(bc2[:], sel2[:], rstd[:], start=True, stop=True)
            nc.vector.tensor_tensor(
                zn16[0:64, 0:8], zc[0:64, :], bc2[0:64, :], op=ALU.mult
            )
            nc.vector.tensor_tensor(
                zn16[64:128, 8:16], zc[64:128, :], bc2[64:128, :], op=ALU.mult
            )

            # --- proj (ln affine folded into wp/bp): lat[128m+p, s] ---
            for m in range(4):
                ps = ps_m.tile([128, 16], f32)
                nc.tensor.matmul(
                    ps[:], wp[:, 128 * m : 128 * (m + 1)], zn16[:],
                    start=True, stop=True,
                )
                nc.vector.tensor_scalar(
                    lat[:, 16 * m : 16 * (m + 1)], ps[:], bp[:, m : m + 1],
                    None, op0=ALU.add,
                )

            # --- h1 = relu(latent @ w1 + b1), transposed ---
            for n in range(2):
                ps = ps_m.tile([128, 16], f32)
                for k in range(4):
                    nc.tensor.matmul(
                        ps[:],
                        w1[:, (k * 2 + n) * 128 : (k * 2 + n + 1) * 128],
                        lat[:, 16 * k : 16 * (k + 1)],
                        start=(k == 0), stop=(k == 3),
                    )
                nc.vector.tensor_scalar(
                    h1[:, 16 * n : 16 * (n + 1)], ps[:], b1[:, n : n + 1],
                    0.0, op0=ALU.add, op1=ALU.max,
                )

            # --- h2 = relu(h1 @ w2 + b2), transposed ---
            for n in range(4):
                ps = ps_m.tile([128, 16], f32)
                for k in range(2):
                    nc.tensor.matmul(
                        ps[:],
                        w2[:, (k * 4 + n) * 128 : (k * 4 + n + 1) * 128],
                        h1[:, 16 * k : 16 * (k + 1)],
                        start=(k == 0), stop=(k == 1),
                    )
                nc.vector.tensor_scalar(
                    h2[:, 16 * n : 16 * (n + 1)], ps[:], b2[:, n : n + 1],
                    0.0, op0=ALU.add, op1=ALU.max,
                )

            # --- out = h2.T @ w3, col-tiled: 4 concurrent M=16 matmuls per
            # PSUM bank (tile_position col groups), copies split across
            # Vector/Scalar engines ---
            for g in range(3):
                psg = ps_o.tile([128, 512], f32, name="pso")
                for sub in range(4):
                    for k in range(4):
                        nc.tensor.matmul(
                            psg[32 * sub : 32 * sub + 16, :],
                            h2[:, 16 * k : 16 * (k + 1)],
                            w3t[:, g, k, 512 * sub : 512 * (sub + 1)],
                            start=(k == 0), stop=(k == 3),
                            tile_position=(0, 32 * sub),
                        )
                # one full-partition copy moves all 4 col-group quarters
                # (garbage rows included; the host slices them off)
                dst = out_sb[:, 512 * g : 512 * (g + 1)]
                if g % 2 == 0:
                    nc.vector.tensor_copy(dst, psg[:])
                else:
                    nc.scalar.copy(dst, psg[:])
                nc.sync.dma_start(
                    out_d[:, 512 * g : 512 * (g + 1)],
                    out_sb[:, 512 * g : 512 * (g + 1)],
                )

    nc.finalize()
    return nc


def _bf16(a):
    import ml_dtypes

    return np.ascontiguousarray(a.astype(ml_dtypes.bfloat16))


def pack_weights(ln_g, ln_b, proj_w, proj_b, w1, b1, w2, b2, w3, b3):
    c = np.ascontiguousarray
    wp = (ln_g[:, None] * proj_w).astype(np.float32)  # [64, 512]
    bpv = (ln_b.astype(np.float64) @ proj_w.astype(np.float64)).astype(np.float32) + proj_b
    return {
        "wp2": _bf16(np.vstack([wp, wp])),
        "bp": c(bpv.reshape(4, 128).T),
        "w1p": _bf16(
            w1.reshape(4, 128, 2, 128).transpose(1, 0, 2, 3).reshape(128, 1024)
        ),
        "b1p": c(b1.reshape(2, 128).T),
        "w2p": _bf16(
            w2.reshape(2, 128, 4, 128).transpose(1, 0, 2, 3).reshape(128, 1024)
        ),
        "b2p": c(b2.reshape(4, 128).T),
        "w3p": _bf16(w3.reshape(4, 128, 3, 2048).transpose(1, 2, 0, 3)),
        "selT": c(np.repeat(np.eye(2, dtype=np.float32), 64, axis=0)),
        "sel": c(np.repeat(np.eye(2, dtype=np.float32), 64, axis=1)),
    }


def pack_feat_core(feat16, feat32, bounds, c):
    xt = np.full((128, 8, S_PAD), F16MIN, np.float16)
    for sl in range(SEGS_PER_CORE):
        seg = c * SEGS_PER_CORE + sl
        a, b = bounds[seg], bounds[seg + 1]
        L = b - a
        if L > S_PAD:
            blk = np.concatenate(
                [
                    feat16[a : a + S_PAD - 1],
                    feat32[a + S_PAD - 1 : b].max(0, keepdims=True).astype(np.float16),
                ],
                0,
            )
            L = S_PAD
        else:
            blk = feat16[a:b]
        g, t = divmod(sl, 8)
        if L > 0:
            xt[g * 64 : (g + 1) * 64, t, :L] = blk.T
    return xt


def make_in_maps(inputs):
    feat32 = np.asarray(inputs["feat"], dtype=np.float32)
    feat16 = feat32.astype(np.float16)
    batch = np.asarray(inputs["batch"])
    wdict = pack_weights(
        *(np.asarray(inputs[k], dtype=np.float32) for k in
          ("ln_g", "ln_b", "proj_w", "proj_b", "w1", "b1", "w2", "b2", "w3", "b3"))
    )
    bounds = np.searchsorted(batch, np.arange(B + 1))
    return [
        {"xt": pack_feat_core(feat16, feat32, bounds, c), **wdict}
        for c in range(N_CORES)
    ]


def kernel(**inputs):
    from concourse.bass_utils import run_bass_kernel_spmd

    if "nc" not in _CACHE:
        _CACHE["nc"] = build_nc()
    nc = _CACHE["nc"]

    in_maps = make_in_maps(inputs)
    res = run_bass_kernel_spmd(nc, in_maps, list(range(N_CORES)))

    out = np.empty((B, OUT_F), np.float32)
    for c in range(N_CORES):
        r = np.asarray(res.results[c]["out"], dtype=np.float32)  # [128, 1536]
        # row 32*sub+s, col 512*g+n -> out[16c+s, 2048g+512sub+n]
        blk = r.reshape(4, 32, 3, 512)[:, :16]  # [sub, s, g, n]
        out[c * 16 : (c + 1) * 16] = blk.transpose(1, 2, 0, 3).reshape(16, OUT_F)
    out += np.asarray(inputs["b3"], dtype=np.float32)[None, :]
    return out.reshape(B, 2048, 3)
